# revision 14
# baseline (speedup 1.0000x reference)
"""Trainium2 Bass kernel for CorrelationVolume + MatchingNet.

Shards the 98 (batch, du, dv) displacement units across 8 NeuronCores
(13/13/12/12/12/12/12/12, padded to a uniform 13 images per core).
Host builds the masked/shifted cost-volume input slabs in a zero-padded
66x98 layout; the device runs the 6-layer conv net (convs as
tap-accumulated fp32r matmuls, instance norm via bn_stats/bn_aggr,
fused leaky-relu apply).
"""

import os
import numpy as np

# problem geometry (hardcoded per contract)
B, C, H, W = 2, 32, 64, 96
HP, WP = H + 2, W + 2          # 66, 98 padded
SP = HP * WP                   # 6468
H2, W2 = H // 2, W // 2        # 32, 48
HP2, WP2 = H2 + 2, W2 + 2      # 34, 50
SP2 = HP2 * WP2                # 1700
NU = B * 49                    # 98 units
NCORES = 8
COUNTS = [13, 13, 12, 12, 12, 12, 12, 12]
OFFS = np.cumsum([0] + COUNTS)[:-1]
IMG = 13

_cache = {}
_last_exec_time_ns = None


def _install_trace_hook():
    """Best-effort install of the axon NTFF profile hook (for BASS_TRACE)."""
    import sys, types
    try:
        import antenv
        if "antenv.axon_hooks" in sys.modules:
            return
        mod = types.ModuleType("antenv.axon_hooks")
        _h = [None]
        mod.set_axon_ntff_profile_hook = lambda h: _h.__setitem__(0, h)
        mod.get_axon_ntff_profile_hook = lambda: _h[0]
        sys.modules["antenv.axon_hooks"] = mod
        antenv.axon_hooks = mod
        from trn_agent_boot.trn_boot import _ntff_profile_via_ctypes
        mod.set_axon_ntff_profile_hook(
            _ntff_profile_via_ctypes('/opt/axon/libaxon_pjrt.so'))
    except Exception:
        pass


def _build_program():
    import concourse.bacc as bacc
    import concourse.mybir as mybir
    from concourse.tile import TileContext

    f32 = mybir.dt.float32
    f32r = mybir.dt.float32r
    AF = mybir.ActivationFunctionType
    ALU = mybir.AluOpType

    nc = bacc.Bacc()

    A0 = nc.declare_dram_parameter("a0", [IMG, 128, SP], f32, isOutput=False)
    W1S = nc.declare_dram_parameter("w1s", [128, 3, 96], f32, isOutput=False)
    W1B = nc.declare_dram_parameter("w1b", [64, 3, 96], f32, isOutput=False)
    W2S = nc.declare_dram_parameter("w2s", [96, 9, 128], f32, isOutput=False)
    W3S = nc.declare_dram_parameter("w3s", [128, 9, 128], f32, isOutput=False)
    W4S = nc.declare_dram_parameter("w4s", [128, 9, 64], f32, isOutput=False)
    W5S = nc.declare_dram_parameter("w5s", [128, 8, 32], f32, isOutput=False)
    W6S = nc.declare_dram_parameter("w6s", [96, 3], f32, isOutput=False)
    B6 = nc.declare_dram_parameter("b6", [1, 1], f32, isOutput=False)
    ZD = nc.declare_dram_parameter("zd", [128, 6600], f32, isOutput=False)
    OUT = nc.declare_dram_parameter("out", [IMG, H * W], f32, isOutput=True)

    with TileContext(nc) as tc:
        with (
            tc.tile_pool(name="wpool", bufs=1) as wpool,
            tc.tile_pool(name="acts", bufs=1) as apool,
            tc.tile_pool(name="a0p", bufs=2) as a0pool,
            tc.tile_pool(name="small", bufs=2) as spool,
            tc.tile_pool(name="outp", bufs=1) as opool,
            tc.tile_pool(name="psum", bufs=3, space="PSUM") as psum,
        ):
            # ---- weights (cast-DMA f32 -> f32r) ----
            w1s = wpool.tile([128, 3, 96], f32r)
            nc.gpsimd.dma_start(out=w1s, in_=W1S[:, :, :])
            w1b = wpool.tile([128, 3, 96], f32r)
            nc.gpsimd.dma_start(out=w1b[64:128, :, :], in_=W1B[:, :, :])
            w2s = wpool.tile([96, 9, 128], f32r)
            nc.gpsimd.dma_start(out=w2s, in_=W2S[:, :, :])
            w3s = wpool.tile([128, 9, 128], f32r)
            nc.gpsimd.dma_start(out=w3s, in_=W3S[:, :, :])
            w4s = wpool.tile([128, 9, 64], f32r)
            nc.gpsimd.dma_start(out=w4s, in_=W4S[:, :, :])
            w5s = wpool.tile([128, 8, 32], f32r)
            nc.gpsimd.dma_start(out=w5s, in_=W5S[:, :, :])
            w6s = wpool.tile([96, 3], f32r)
            nc.gpsimd.dma_start(out=w6s, in_=W6S[:, :])
            b6t = wpool.tile([1, 1], f32)
            nc.sync.dma_start(out=b6t, in_=B6[:, :])
            ep = wpool.tile([128, 1], f32)
            nc.vector.memset(ep, 1e-5)

            # ---- persistent activation buffers (pad rings stay zero) ----
            a1 = apool.tile([96, 6600], f32r)
            a2 = apool.tile([128, SP2], f32r)
            a3 = apool.tile([128, SP2], f32r)
            a4 = apool.tile([128, SP2], f32r)
            a5 = apool.tile([96, SP], f32r)
            # zero act buffers (pad rings) via cast-DMA — fp32r-rounded producer
            for t, npart, width in ((a1, 96, 6600), (a2, 128, SP2),
                                    (a3, 128, SP2), (a4, 128, SP2),
                                    (a5, 96, SP)):
                nc.gpsimd.dma_start(out=t, in_=ZD[0:npart, 0:width])

            va1 = a1[:, :].rearrange("p (r c) -> p r c", c=100)
            va2 = a2[:, :].rearrange("p (r c) -> p r c", c=WP2)
            va3 = a3[:, :].rearrange("p (r c) -> p r c", c=WP2)
            va4 = a4[:, :].rearrange("p (r c) -> p r c", c=WP2)
            va5 = a5[:, :].rearrange("p (r c) -> p r c", c=WP)

            def flat(ap3):
                return ap3[:, :, :].rearrange("p a b -> p (a b)")

            def norm_consts(CC, st):
                """bn_aggr st -> (rstd, shift) for the fused Lrelu apply."""
                mv = spool.tile([CC, 2], f32, tag="mv")
                nc.vector.bn_aggr(out=mv, in_=st)
                rs = spool.tile([CC, 1], f32, tag="rs")
                nc.scalar.activation(out=rs, in_=mv[:, 1:2], func=AF.Sqrt,
                                     bias=ep[0:CC, :], scale=1.0)
                nc.vector.reciprocal(out=rs, in_=rs)
                sh = spool.tile([CC, 1], f32, tag="sh")
                nc.vector.tensor_scalar(out=sh, in0=mv[:, 0:1], scalar1=rs,
                                        scalar2=-1.0, op0=ALU.mult,
                                        op1=ALU.mult)
                return rs, sh

            def stats_apply(CC, st, interior, chunks):
                rs, sh = norm_consts(CC, st)
                for (lo, hi) in chunks:
                    ap = interior(lo, hi)
                    nc.scalar.activation(out=ap, in_=ap, func=AF.Lrelu,
                                         bias=sh, scale=rs, alpha=0.01)
                return rs, sh

            def make_conv1(i):
                """Emit a0 DMA now; return (16 tile thunks, finalize thunk).

                conv1 of image i is interleaved into image i-1's layer
                transitions so PE has work during the norm chains."""
                a0 = a0pool.tile([128, SP], f32r, tag="a0")
                nc.gpsimd.dma_start(out=a0, in_=A0[i, :, :])
                va0 = a0[:, :].rearrange("p (r c) -> p r c", c=WP)
                st1 = spool.tile([96, 13, 6], f32, tag="st1")

                def tile_thunk(t):
                    y0 = t * 5
                    nr = 5 if t < 12 else 4
                    pt = psum.tile([96, nr, 96], f32, tag="mm")
                    for kx in range(3):
                        nc.tensor.matmul(pt, w1s[0:128, kx, :],
                                         va0[0:128, y0:y0 + nr, kx:kx + 96],
                                         start=(kx == 0), stop=False)
                    for kx in range(3):
                        nc.tensor.matmul(pt, w1b[64:128, kx, :],
                                         va0[64:128, y0 + 1:y0 + 1 + nr,
                                             kx:kx + 96],
                                         start=False, stop=(kx == 2))
                    nc.vector.bn_stats(out=st1[:, t, :], in_=flat(pt))
                    # split raw conv1 into even/odd column planes:
                    # E[x']=col 2x' at plane cols 0-49, O[x']=col 2x'+1 at 50-99
                    nc.vector.tensor_copy(
                        out=va1[0:96, 1 + y0:1 + y0 + nr, 1:49],
                        in_=pt[:, :, 1:96:2])
                    nc.vector.tensor_copy(
                        out=va1[0:96, 1 + y0:1 + y0 + nr, 50:98],
                        in_=pt[:, :, 0:96:2])

                def fin():
                    stats_apply(
                        96, st1,
                        lambda lo, hi: va1[0:96, lo:hi, 1:98],
                        [(1, 9), (9, 17), (17, 33), (33, 49), (49, 65)])

                return [lambda t=t: tile_thunk(t) for t in range(13)], fin

            # image 0's conv1 runs un-overlapped
            cur_tiles, cur_fin = make_conv1(0)
            for th in cur_tiles:
                th()
            cur_fin()

            for i in range(IMG):
                if i + 1 < IMG:
                    nxt_tiles, nxt_fin = make_conv1(i + 1)
                else:
                    nxt_tiles, nxt_fin = [], (lambda: None)

                # ---- conv2: 96 -> 128, 3x3 stride 2 -> 32x48 ----
                st2 = spool.tile([128, 4, 6], f32, tag="st2")
                pts = []
                for t in range(4):
                    y0 = t * 8
                    pt = psum.tile([128, 8, 48], f32, tag="c234", bufs=5)
                    for idx in range(9):
                        ky, kx = divmod(idx, 3)
                        co = (0, 50, 1)[kx]
                        rhs = va1[0:96, 2 * y0 + ky:2 * y0 + ky + 16:2,
                                  co:co + 48]
                        nc.tensor.matmul(pt, w2s[0:96, idx, :], rhs,
                                         start=(idx == 0), stop=(idx == 8))
                    nc.vector.bn_stats(out=st2[:, t, :], in_=flat(pt))
                    pts.append(pt)
                for th in nxt_tiles[0:4]:
                    th()
                rs, sh = norm_consts(128, st2)
                for t, pt in enumerate(pts):
                    y0 = t * 8
                    nc.scalar.activation(
                        out=va2[0:128, 1 + y0:1 + y0 + 8, 1:49], in_=pt,
                        func=AF.Lrelu, bias=sh, scale=rs, alpha=0.01)

                # ---- conv3: 128 -> 128, 3x3, on 34x50 padded ----
                st3 = spool.tile([128, 4, 6], f32, tag="st3")
                pts = []
                for t in range(4):
                    y0 = t * 8
                    pt = psum.tile([128, 8, 48], f32, tag="c234", bufs=5)
                    for idx in range(9):
                        ky, kx = divmod(idx, 3)
                        rhs = va2[0:128, y0 + ky:y0 + ky + 8, kx:kx + 48]
                        nc.tensor.matmul(pt, w3s[0:128, idx, :], rhs,
                                         start=(idx == 0), stop=(idx == 8))
                    nc.vector.bn_stats(out=st3[:, t, :], in_=flat(pt))
                    pts.append(pt)
                for th in nxt_tiles[4:8]:
                    th()
                rs, sh = norm_consts(128, st3)
                for t, pt in enumerate(pts):
                    y0 = t * 8
                    nc.scalar.activation(
                        out=va3[0:128, 1 + y0:1 + y0 + 8, 1:49], in_=pt,
                        func=AF.Lrelu, bias=sh, scale=rs, alpha=0.01)

                # ---- conv4: 128 -> 64, 3x3 ----
                st4 = spool.tile([64, 4, 6], f32, tag="st4")
                pts = []
                for t in range(4):
                    y0 = t * 8
                    pt = psum.tile([64, 8, 48], f32, tag="c234", bufs=5)
                    for idx in range(9):
                        ky, kx = divmod(idx, 3)
                        rhs = va3[0:128, y0 + ky:y0 + ky + 8, kx:kx + 48]
                        nc.tensor.matmul(pt, w4s[0:128, idx, :], rhs,
                                         start=(idx == 0), stop=(idx == 8))
                    nc.vector.bn_stats(out=st4[:, t, :], in_=flat(pt))
                    pts.append(pt)
                for th in nxt_tiles[8:13]:
                    th()
                rs, sh = norm_consts(64, st4)
                for t, pt in enumerate(pts):
                    y0 = t * 8
                    lo = 1 + y0
                    nc.scalar.activation(
                        out=va4[0:64, lo:lo + 8, 1:49], in_=pt,
                        func=AF.Lrelu, bias=sh, scale=rs, alpha=0.01)
                    # dup applied chunk shifted one padded row to parts 64-127
                    nc.sync.dma_start(
                        out=a4[64:128, (lo - 1) * WP2:(lo + 7) * WP2],
                        in_=a4[0:64, lo * WP2:(lo + 8) * WP2])

                # ---- deconv: 64 -> 32, 4x4 stride 2 -> 64x96 (4 parities) ----
                st5 = spool.tile([32, 16, 6], f32, tag="st5")
                for py in range(2):
                    for px in range(2):
                        for t in range(4):
                            r0 = t * 8
                            pt = psum.tile([32, 8, 48], f32, tag="c234",
                                           bufs=5)
                            for cx in range(2):
                                rhs = va4[0:128, r0 + py:r0 + py + 8,
                                          px + cx:px + cx + 48]
                                nc.tensor.matmul(
                                    pt, w5s[0:128, (py * 2 + px) * 2 + cx, :],
                                    rhs, start=(cx == 0), stop=(cx == 1))
                            qi = (py * 2 + px) * 4 + t
                            nc.vector.bn_stats(out=st5[:, qi, :], in_=flat(pt))
                            dst = va5[0:32,
                                      1 + 2 * r0 + py:1 + 2 * r0 + py + 16:2,
                                      1 + px:1 + px + 96:2]
                            if qi % 2 == 0:
                                nc.vector.tensor_copy(out=dst, in_=pt)
                            else:
                                nc.scalar.activation(out=dst, in_=pt,
                                                     func=AF.Copy)
                nxt_fin()
                # apply + chunk-interleaved replication at +1/+2 padded rows
                rs, sh = norm_consts(32, st5)
                for r in range(0, 64, 16):
                    lo = 1 + r
                    ap = va5[0:32, lo:lo + 16, 1:97]
                    nc.scalar.activation(out=ap, in_=ap, func=AF.Lrelu,
                                         bias=sh, scale=rs, alpha=0.01)
                    nc.sync.dma_start(
                        out=a5[32:64, (lo - 1) * WP:(lo + 15) * WP],
                        in_=a5[0:32, lo * WP:(lo + 16) * WP])
                    b2lo = max(lo - 2, 0)
                    nc.sync.dma_start(
                        out=a5[64:96, b2lo * WP:(lo + 14) * WP],
                        in_=a5[0:32, (b2lo + 2) * WP:(lo + 16) * WP])

                # ---- conv6: 32 -> 1, 3x3 (+bias) ----
                ot = opool.tile([1, H * W], f32, tag="ot")
                for t in range(13):
                    y0 = t * 5
                    nr = 5 if t < 12 else 4
                    pt = psum.tile([1, nr, 96], f32, tag="mm")
                    for kx in range(3):
                        rhs = va5[0:96, y0:y0 + nr, kx:kx + 96]
                        nc.tensor.matmul(pt, w6s[0:96, kx:kx + 1], rhs,
                                         start=(kx == 0), stop=(kx == 2))
                    nc.scalar.activation(
                        out=ot[0:1, y0 * 96:(y0 + nr) * 96],
                        in_=flat(pt), func=AF.Identity,
                        bias=b6t[0:1, :], scale=1.0)
                nc.sync.dma_start(out=OUT[i:i + 1, :], in_=ot)

    nc.finalize()
    return nc


def _host_inputs(fmap1, fmap2, w1, w2, w3, w4, w5, w6, b6):
    fmap1 = np.asarray(fmap1, np.float32)
    fmap2 = np.asarray(fmap2, np.float32)
    w1 = np.asarray(w1, np.float32)
    w2 = np.asarray(w2, np.float32)
    w3 = np.asarray(w3, np.float32)
    w4 = np.asarray(w4, np.float32)
    w5 = np.asarray(w5, np.float32)
    w6 = np.asarray(w6, np.float32)
    b6 = np.asarray(b6, np.float32)

    # per-unit padded input slabs
    slabs = np.zeros((NCORES, IMG, 128, HP, WP), np.float32)
    for u in range(NU):
        bi, r = divmod(u, 49)
        di, dj = r // 7 - 3, r % 7 - 3
        y0, y1 = max(0, -dj), min(H, H - dj)
        x0, x1 = max(0, -di), min(W, W - di)
        k = np.searchsorted(OFFS, u, side="right") - 1
        s = u - OFFS[k]
        sl = slabs[k, s]
        sl[0:32, 1 + y0:1 + y1, 1 + x0:1 + x1] = fmap1[bi, :, y0:y1, x0:x1]
        sl[32:64, 1 + y0:1 + y1, 1 + x0:1 + x1] = \
            fmap2[bi, :, y0 + dj:y1 + dj, x0 + di:x1 + di]
        sl[64:128, 0:HP - 1, :] = sl[0:64, 1:HP, :]

    # weight banks (lhsT layouts, K on partitions)
    w1s = np.zeros((128, 3, 96), np.float32)
    w1b = np.zeros((64, 3, 96), np.float32)
    for kx in range(3):
        w1s[0:64, kx] = w1[:, :, 0, kx].T
        w1s[64:128, kx] = w1[:, :, 1, kx].T
        w1b[:, kx] = w1[:, :, 2, kx].T
    w2s = np.zeros((96, 9, 128), np.float32)
    w3s = np.zeros((128, 9, 128), np.float32)
    w4s = np.zeros((128, 9, 64), np.float32)
    for idx in range(9):
        ky, kx = divmod(idx, 3)
        w2s[:, idx] = w2[:, :, ky, kx].T
        w3s[:, idx] = w3[:, :, ky, kx].T
        w4s[:, idx] = w4[:, :, ky, kx].T
    wf = np.flip(w5, (2, 3)).transpose(1, 0, 2, 3)  # [out=32, in=64, 4, 4]
    w5s = np.zeros((128, 8, 32), np.float32)
    for py in range(2):
        for px in range(2):
            for cx in range(2):
                col = (py * 2 + px) * 2 + cx
                w5s[0:64, col] = wf[:, :, py, px + 2 * cx].T
                w5s[64:128, col] = wf[:, :, py + 2, px + 2 * cx].T
    w6s = np.zeros((96, 3), np.float32)
    for kx in range(3):
        for pb in range(3):
            w6s[32 * pb:32 * pb + 32, kx] = w6[0, :, pb, kx]
    b6r = b6.reshape(1, 1)

    in_maps = []
    for k in range(NCORES):
        in_maps.append({
            "a0": slabs[k].reshape(IMG, 128, SP),
            "w1s": w1s, "w1b": w1b, "w2s": w2s, "w3s": w3s, "w4s": w4s,
            "w5s": w5s, "w6s": w6s, "b6": b6r,
            "zd": np.zeros((128, 6600), np.float32),
        })
    return in_maps


def kernel(fmap1, fmap2, w1, w2, w3, w4, w5, w6, b6):
    global _last_exec_time_ns
    if os.environ.get("BASS_TRACE"):
        _install_trace_hook()
    from concourse.bass_utils import run_bass_kernel_spmd

    if "nc" not in _cache:
        _cache["nc"] = _build_program()
    nc = _cache["nc"]

    in_maps = _host_inputs(fmap1, fmap2, w1, w2, w3, w4, w5, w6, b6)

    last_err = None
    for _ in range(3):
        try:
            res = run_bass_kernel_spmd(nc, in_maps, list(range(NCORES)))
            break
        except Exception as e:  # transient device/runtime hiccups
            last_err = e
    else:
        raise last_err
    _last_exec_time_ns = res.exec_time_ns

    out = np.zeros((NU, H * W), np.float32)
    for k in range(NCORES):
        out[OFFS[k]:OFFS[k] + COUNTS[k]] = \
            res.results[k]["out"][:COUNTS[k]]
    return out.reshape(B, 7, 7, H, W)


# revision 15
# speedup vs baseline: 1.0125x; 1.0125x over previous
"""Trainium2 Bass kernel for CorrelationVolume + MatchingNet.

Shards the 98 (batch, du, dv) displacement units across 8 NeuronCores
(13/13/12/12/12/12/12/12, padded to a uniform 13 images per core).
Host builds the masked/shifted cost-volume input slabs in a zero-padded
66x98 layout; the device runs the 6-layer conv net (convs as
tap-accumulated fp32r matmuls, instance norm via bn_stats/bn_aggr,
fused leaky-relu apply).
"""

import os
import numpy as np

# problem geometry (hardcoded per contract)
B, C, H, W = 2, 32, 64, 96
HP, WP = H + 2, W + 2          # 66, 98 padded
SP = HP * WP                   # 6468
H2, W2 = H // 2, W // 2        # 32, 48
HP2, WP2 = H2 + 2, W2 + 2      # 34, 50
SP2 = HP2 * WP2                # 1700
NU = B * 49                    # 98 units
NCORES = 8
COUNTS = [13, 13, 12, 12, 12, 12, 12, 12]
OFFS = np.cumsum([0] + COUNTS)[:-1]
IMG = 13

_cache = {}
_last_exec_time_ns = None


def _install_trace_hook():
    """Best-effort install of the axon NTFF profile hook (for BASS_TRACE)."""
    import sys, types
    try:
        import antenv
        if "antenv.axon_hooks" in sys.modules:
            return
        mod = types.ModuleType("antenv.axon_hooks")
        _h = [None]
        mod.set_axon_ntff_profile_hook = lambda h: _h.__setitem__(0, h)
        mod.get_axon_ntff_profile_hook = lambda: _h[0]
        sys.modules["antenv.axon_hooks"] = mod
        antenv.axon_hooks = mod
        from trn_agent_boot.trn_boot import _ntff_profile_via_ctypes
        mod.set_axon_ntff_profile_hook(
            _ntff_profile_via_ctypes('/opt/axon/libaxon_pjrt.so'))
    except Exception:
        pass


def _build_program():
    import concourse.bacc as bacc
    import concourse.mybir as mybir
    from concourse.tile import TileContext

    f32 = mybir.dt.float32
    f32r = mybir.dt.float32r
    AF = mybir.ActivationFunctionType
    ALU = mybir.AluOpType

    nc = bacc.Bacc()

    A0 = nc.declare_dram_parameter("a0", [IMG, 128, SP], f32, isOutput=False)
    W1S = nc.declare_dram_parameter("w1s", [128, 3, 96], f32, isOutput=False)
    W1B = nc.declare_dram_parameter("w1b", [64, 3, 96], f32, isOutput=False)
    W2S = nc.declare_dram_parameter("w2s", [96, 9, 128], f32, isOutput=False)
    W3S = nc.declare_dram_parameter("w3s", [128, 9, 128], f32, isOutput=False)
    W4S = nc.declare_dram_parameter("w4s", [128, 9, 64], f32, isOutput=False)
    W5S = nc.declare_dram_parameter("w5s", [128, 8, 32], f32, isOutput=False)
    W6S = nc.declare_dram_parameter("w6s", [96, 3], f32, isOutput=False)
    B6 = nc.declare_dram_parameter("b6", [1, 1], f32, isOutput=False)
    ZD = nc.declare_dram_parameter("zd", [128, 6600], f32, isOutput=False)
    OUT = nc.declare_dram_parameter("out", [IMG, H * W], f32, isOutput=True)

    with TileContext(nc) as tc:
        with (
            tc.tile_pool(name="wpool", bufs=1) as wpool,
            tc.tile_pool(name="acts", bufs=1) as apool,
            tc.tile_pool(name="a0p", bufs=2) as a0pool,
            tc.tile_pool(name="small", bufs=2) as spool,
            tc.tile_pool(name="outp", bufs=1) as opool,
            tc.tile_pool(name="psum", bufs=3, space="PSUM") as psum,
        ):
            # ---- weights (cast-DMA f32 -> f32r) ----
            w1s = wpool.tile([128, 3, 96], f32r)
            nc.gpsimd.dma_start(out=w1s, in_=W1S[:, :, :])
            w1b = wpool.tile([128, 3, 96], f32r)
            nc.gpsimd.dma_start(out=w1b[64:128, :, :], in_=W1B[:, :, :])
            w2s = wpool.tile([96, 9, 128], f32r)
            nc.gpsimd.dma_start(out=w2s, in_=W2S[:, :, :])
            w3s = wpool.tile([128, 9, 128], f32r)
            nc.gpsimd.dma_start(out=w3s, in_=W3S[:, :, :])
            w4s = wpool.tile([128, 9, 64], f32r)
            nc.gpsimd.dma_start(out=w4s, in_=W4S[:, :, :])
            w5s = wpool.tile([128, 8, 32], f32r)
            nc.gpsimd.dma_start(out=w5s, in_=W5S[:, :, :])
            w6s = wpool.tile([96, 3], f32r)
            nc.gpsimd.dma_start(out=w6s, in_=W6S[:, :])
            b6t = wpool.tile([1, 1], f32)
            nc.sync.dma_start(out=b6t, in_=B6[:, :])
            ep = wpool.tile([128, 1], f32)
            nc.vector.memset(ep, 1e-5)

            # ---- persistent activation buffers (pad rings stay zero) ----
            a1 = apool.tile([96, 6600], f32r)
            a2 = apool.tile([128, SP2], f32r)
            a3 = apool.tile([128, SP2], f32r)
            a4 = apool.tile([128, SP2], f32r)
            a5 = apool.tile([96, SP], f32r)
            # zero act buffers (pad rings) via cast-DMA — fp32r-rounded producer
            for t, npart, width in ((a1, 96, 6600), (a2, 128, SP2),
                                    (a3, 128, SP2), (a4, 128, SP2),
                                    (a5, 96, SP)):
                nc.gpsimd.dma_start(out=t, in_=ZD[0:npart, 0:width])

            va1 = a1[:, :].rearrange("p (r c) -> p r c", c=100)
            va2 = a2[:, :].rearrange("p (r c) -> p r c", c=WP2)
            va3 = a3[:, :].rearrange("p (r c) -> p r c", c=WP2)
            va4 = a4[:, :].rearrange("p (r c) -> p r c", c=WP2)
            va5 = a5[:, :].rearrange("p (r c) -> p r c", c=WP)

            def flat(ap3):
                return ap3[:, :, :].rearrange("p a b -> p (a b)")

            def norm_consts(CC, st):
                """bn_aggr st -> (rstd, shift) for the fused Lrelu apply."""
                mv = spool.tile([CC, 2], f32, tag="mv")
                nc.vector.bn_aggr(out=mv, in_=st)
                rs = spool.tile([CC, 1], f32, tag="rs")
                nc.scalar.activation(out=rs, in_=mv[:, 1:2], func=AF.Sqrt,
                                     bias=ep[0:CC, :], scale=1.0)
                nc.vector.reciprocal(out=rs, in_=rs)
                sh = spool.tile([CC, 1], f32, tag="sh")
                nc.vector.tensor_scalar(out=sh, in0=mv[:, 0:1], scalar1=rs,
                                        scalar2=-1.0, op0=ALU.mult,
                                        op1=ALU.mult)
                return rs, sh

            def stats_apply(CC, st, interior, chunks):
                rs, sh = norm_consts(CC, st)
                for (lo, hi) in chunks:
                    ap = interior(lo, hi)
                    nc.scalar.activation(out=ap, in_=ap, func=AF.Lrelu,
                                         bias=sh, scale=rs, alpha=0.01)
                return rs, sh

            def make_conv1(i):
                """Emit a0 DMA now; return (16 tile thunks, finalize thunk).

                conv1 of image i is interleaved into image i-1's layer
                transitions so PE has work during the norm chains."""
                a0 = a0pool.tile([128, SP], f32r, tag="a0")
                nc.gpsimd.dma_start(out=a0, in_=A0[i, :, :])
                va0 = a0[:, :].rearrange("p (r c) -> p r c", c=WP)
                st1 = spool.tile([96, 13, 6], f32, tag="st1")

                def tile_thunk(t):
                    y0 = t * 5
                    nr = 5 if t < 12 else 4
                    pt = psum.tile([96, nr, 96], f32, tag="mm")
                    for kx in range(3):
                        nc.tensor.matmul(pt, w1s[0:128, kx, :],
                                         va0[0:128, y0:y0 + nr, kx:kx + 96],
                                         start=(kx == 0), stop=False)
                    for kx in range(3):
                        nc.tensor.matmul(pt, w1b[64:128, kx, :],
                                         va0[64:128, y0 + 1:y0 + 1 + nr,
                                             kx:kx + 96],
                                         start=False, stop=(kx == 2))
                    nc.vector.bn_stats(out=st1[:, t, :], in_=flat(pt))
                    # split raw conv1 into even/odd column planes:
                    # E[x']=col 2x' at plane cols 0-49, O[x']=col 2x'+1 at 50-99
                    nc.vector.tensor_copy(
                        out=va1[0:96, 1 + y0:1 + y0 + nr, 1:49],
                        in_=pt[:, :, 1:96:2])
                    nc.vector.tensor_copy(
                        out=va1[0:96, 1 + y0:1 + y0 + nr, 50:98],
                        in_=pt[:, :, 0:96:2])

                def fin():
                    stats_apply(
                        96, st1,
                        lambda lo, hi: va1[0:96, lo:hi, 1:98],
                        [(1, 9), (9, 17), (17, 33), (33, 49), (49, 65)])

                return [lambda t=t: tile_thunk(t) for t in range(13)], fin

            # image 0's conv1 runs un-overlapped
            cur_tiles, cur_fin = make_conv1(0)
            for th in cur_tiles:
                th()
            cur_fin()

            for i in range(IMG):
                if i + 1 < IMG:
                    nxt_tiles, nxt_fin = make_conv1(i + 1)
                else:
                    nxt_tiles, nxt_fin = [], (lambda: None)

                # ---- conv2: 96 -> 128, 3x3 stride 2 -> 32x48 ----
                st2 = spool.tile([128, 4, 6], f32, tag="st2")
                pts = []
                for t in range(4):
                    y0 = t * 8
                    pt = psum.tile([128, 8, 48], f32, tag="c234", bufs=5)
                    for idx in range(9):
                        ky, kx = divmod(idx, 3)
                        co = (0, 50, 1)[kx]
                        rhs = va1[0:96, 2 * y0 + ky:2 * y0 + ky + 16:2,
                                  co:co + 48]
                        nc.tensor.matmul(pt, w2s[0:96, idx, :], rhs,
                                         start=(idx == 0), stop=(idx == 8))
                    nc.vector.bn_stats(out=st2[:, t, :], in_=flat(pt))
                    pts.append(pt)
                for th in nxt_tiles[0:4]:
                    th()
                rs, sh = norm_consts(128, st2)
                for t, pt in enumerate(pts):
                    y0 = t * 8
                    nc.scalar.activation(
                        out=va2[0:128, 1 + y0:1 + y0 + 8, 1:49], in_=pt,
                        func=AF.Lrelu, bias=sh, scale=rs, alpha=0.01)

                # ---- conv3: 128 -> 128, 3x3, on 34x50 padded ----
                st3 = spool.tile([128, 4, 6], f32, tag="st3")
                pts = []
                for t in range(4):
                    y0 = t * 8
                    pt = psum.tile([128, 8, 48], f32, tag="c234", bufs=5)
                    for idx in range(9):
                        ky, kx = divmod(idx, 3)
                        rhs = va2[0:128, y0 + ky:y0 + ky + 8, kx:kx + 48]
                        nc.tensor.matmul(pt, w3s[0:128, idx, :], rhs,
                                         start=(idx == 0), stop=(idx == 8))
                    nc.vector.bn_stats(out=st3[:, t, :], in_=flat(pt))
                    pts.append(pt)
                for th in nxt_tiles[4:8]:
                    th()
                rs, sh = norm_consts(128, st3)
                for t, pt in enumerate(pts):
                    y0 = t * 8
                    nc.scalar.activation(
                        out=va3[0:128, 1 + y0:1 + y0 + 8, 1:49], in_=pt,
                        func=AF.Lrelu, bias=sh, scale=rs, alpha=0.01)

                # ---- conv4: 128 -> 64, 3x3 ----
                st4 = spool.tile([64, 4, 6], f32, tag="st4")
                pts = []
                for t in range(4):
                    y0 = t * 8
                    pt = psum.tile([64, 8, 48], f32, tag="c234", bufs=5)
                    for idx in range(9):
                        ky, kx = divmod(idx, 3)
                        rhs = va3[0:128, y0 + ky:y0 + ky + 8, kx:kx + 48]
                        nc.tensor.matmul(pt, w4s[0:128, idx, :], rhs,
                                         start=(idx == 0), stop=(idx == 8))
                    nc.vector.bn_stats(out=st4[:, t, :], in_=flat(pt))
                    pts.append(pt)
                for th in nxt_tiles[6:10]:
                    th()
                rs, sh = norm_consts(64, st4)
                for t, pt in enumerate(pts):
                    y0 = t * 8
                    lo = 1 + y0
                    nc.scalar.activation(
                        out=va4[0:64, lo:lo + 8, 1:49], in_=pt,
                        func=AF.Lrelu, bias=sh, scale=rs, alpha=0.01)
                    # dup applied chunk shifted one padded row to parts 64-127
                    nc.sync.dma_start(
                        out=a4[64:128, (lo - 1) * WP2:(lo + 7) * WP2],
                        in_=a4[0:64, lo * WP2:(lo + 8) * WP2])

                # ---- deconv: 64 -> 32, 4x4 stride 2 -> 64x96 (4 parities) ----
                st5 = spool.tile([32, 16, 6], f32, tag="st5")
                for py, px in ((0, 0), (0, 1), (None, None), (1, 0), (1, 1)):
                    if py is None:
                        for th in nxt_tiles[10:13]:
                            th()
                        nxt_fin()
                        continue
                    if True:
                        for t in range(4):
                            r0 = t * 8
                            pt = psum.tile([32, 8, 48], f32, tag="c234",
                                           bufs=5)
                            for cx in range(2):
                                rhs = va4[0:128, r0 + py:r0 + py + 8,
                                          px + cx:px + cx + 48]
                                nc.tensor.matmul(
                                    pt, w5s[0:128, (py * 2 + px) * 2 + cx, :],
                                    rhs, start=(cx == 0), stop=(cx == 1))
                            qi = (py * 2 + px) * 4 + t
                            nc.vector.bn_stats(out=st5[:, qi, :], in_=flat(pt))
                            dst = va5[0:32,
                                      1 + 2 * r0 + py:1 + 2 * r0 + py + 16:2,
                                      1 + px:1 + px + 96:2]
                            if qi % 2 == 0:
                                nc.vector.tensor_copy(out=dst, in_=pt)
                            else:
                                nc.scalar.activation(out=dst, in_=pt,
                                                     func=AF.Copy)
                # apply + chunk-interleaved replication at +1/+2 padded rows
                rs, sh = norm_consts(32, st5)
                for r in range(0, 64, 16):
                    lo = 1 + r
                    ap = va5[0:32, lo:lo + 16, 1:97]
                    nc.scalar.activation(out=ap, in_=ap, func=AF.Lrelu,
                                         bias=sh, scale=rs, alpha=0.01)
                    nc.sync.dma_start(
                        out=a5[32:64, (lo - 1) * WP:(lo + 15) * WP],
                        in_=a5[0:32, lo * WP:(lo + 16) * WP])
                    b2lo = max(lo - 2, 0)
                    nc.sync.dma_start(
                        out=a5[64:96, b2lo * WP:(lo + 14) * WP],
                        in_=a5[0:32, (b2lo + 2) * WP:(lo + 16) * WP])

                # ---- conv6: 32 -> 1, 3x3 (+bias) ----
                ot = opool.tile([1, H * W], f32, tag="ot")
                for t in range(13):
                    y0 = t * 5
                    nr = 5 if t < 12 else 4
                    pt = psum.tile([1, nr, 96], f32, tag="mm")
                    for kx in range(3):
                        rhs = va5[0:96, y0:y0 + nr, kx:kx + 96]
                        nc.tensor.matmul(pt, w6s[0:96, kx:kx + 1], rhs,
                                         start=(kx == 0), stop=(kx == 2))
                    nc.scalar.activation(
                        out=ot[0:1, y0 * 96:(y0 + nr) * 96],
                        in_=flat(pt), func=AF.Identity,
                        bias=b6t[0:1, :], scale=1.0)
                nc.sync.dma_start(out=OUT[i:i + 1, :], in_=ot)

    nc.finalize()
    return nc


def _host_inputs(fmap1, fmap2, w1, w2, w3, w4, w5, w6, b6):
    fmap1 = np.asarray(fmap1, np.float32)
    fmap2 = np.asarray(fmap2, np.float32)
    w1 = np.asarray(w1, np.float32)
    w2 = np.asarray(w2, np.float32)
    w3 = np.asarray(w3, np.float32)
    w4 = np.asarray(w4, np.float32)
    w5 = np.asarray(w5, np.float32)
    w6 = np.asarray(w6, np.float32)
    b6 = np.asarray(b6, np.float32)

    # per-unit padded input slabs
    slabs = np.zeros((NCORES, IMG, 128, HP, WP), np.float32)
    for u in range(NU):
        bi, r = divmod(u, 49)
        di, dj = r // 7 - 3, r % 7 - 3
        y0, y1 = max(0, -dj), min(H, H - dj)
        x0, x1 = max(0, -di), min(W, W - di)
        k = np.searchsorted(OFFS, u, side="right") - 1
        s = u - OFFS[k]
        sl = slabs[k, s]
        sl[0:32, 1 + y0:1 + y1, 1 + x0:1 + x1] = fmap1[bi, :, y0:y1, x0:x1]
        sl[32:64, 1 + y0:1 + y1, 1 + x0:1 + x1] = \
            fmap2[bi, :, y0 + dj:y1 + dj, x0 + di:x1 + di]
        sl[64:128, 0:HP - 1, :] = sl[0:64, 1:HP, :]

    # weight banks (lhsT layouts, K on partitions)
    w1s = np.zeros((128, 3, 96), np.float32)
    w1b = np.zeros((64, 3, 96), np.float32)
    for kx in range(3):
        w1s[0:64, kx] = w1[:, :, 0, kx].T
        w1s[64:128, kx] = w1[:, :, 1, kx].T
        w1b[:, kx] = w1[:, :, 2, kx].T
    w2s = np.zeros((96, 9, 128), np.float32)
    w3s = np.zeros((128, 9, 128), np.float32)
    w4s = np.zeros((128, 9, 64), np.float32)
    for idx in range(9):
        ky, kx = divmod(idx, 3)
        w2s[:, idx] = w2[:, :, ky, kx].T
        w3s[:, idx] = w3[:, :, ky, kx].T
        w4s[:, idx] = w4[:, :, ky, kx].T
    wf = np.flip(w5, (2, 3)).transpose(1, 0, 2, 3)  # [out=32, in=64, 4, 4]
    w5s = np.zeros((128, 8, 32), np.float32)
    for py in range(2):
        for px in range(2):
            for cx in range(2):
                col = (py * 2 + px) * 2 + cx
                w5s[0:64, col] = wf[:, :, py, px + 2 * cx].T
                w5s[64:128, col] = wf[:, :, py + 2, px + 2 * cx].T
    w6s = np.zeros((96, 3), np.float32)
    for kx in range(3):
        for pb in range(3):
            w6s[32 * pb:32 * pb + 32, kx] = w6[0, :, pb, kx]
    b6r = b6.reshape(1, 1)

    in_maps = []
    for k in range(NCORES):
        in_maps.append({
            "a0": slabs[k].reshape(IMG, 128, SP),
            "w1s": w1s, "w1b": w1b, "w2s": w2s, "w3s": w3s, "w4s": w4s,
            "w5s": w5s, "w6s": w6s, "b6": b6r,
            "zd": np.zeros((128, 6600), np.float32),
        })
    return in_maps


def kernel(fmap1, fmap2, w1, w2, w3, w4, w5, w6, b6):
    global _last_exec_time_ns
    if os.environ.get("BASS_TRACE"):
        _install_trace_hook()
    from concourse.bass_utils import run_bass_kernel_spmd

    if "nc" not in _cache:
        _cache["nc"] = _build_program()
    nc = _cache["nc"]

    in_maps = _host_inputs(fmap1, fmap2, w1, w2, w3, w4, w5, w6, b6)

    last_err = None
    for _ in range(3):
        try:
            res = run_bass_kernel_spmd(nc, in_maps, list(range(NCORES)))
            break
        except Exception as e:  # transient device/runtime hiccups
            last_err = e
    else:
        raise last_err
    _last_exec_time_ns = res.exec_time_ns

    out = np.zeros((NU, H * W), np.float32)
    for k in range(NCORES):
        out[OFFS[k]:OFFS[k] + COUNTS[k]] = \
            res.results[k]["out"][:COUNTS[k]]
    return out.reshape(B, 7, 7, H, W)
